# revision 9
# baseline (speedup 1.0000x reference)
"""Trainium2 Bass kernel: fractional Brownian motion kernel layer, v2.

K[i,j] = 0.5 * sum_d (|x_id|^p + |X2_jd|^p - |x_id - X2_jd|^p),
p = 2*softplus(log_H),  x:[2048,16], X2:[2048,16] -> K:[2048,2048] f32.

Algorithm: trig-feature factorization. |t|^p is fit (host-side, per call,
ridge-weighted LSQ; frequencies pre-optimized offline for p=1.7, weights
re-fit for the runtime p) as W0 + sum_q w_q cos(om_q t), so the pairwise
term factors exactly:

  cos(om(x-y)) = cos(om x)cos(om y) + sin(om x)sin(om y)

and the whole O(N*M*D) pairwise pow collapses into fp16 matmuls over
C = D*2*Q = 256 feature rows (2 groups of 128) on the otherwise idle
TensorEngine. t1/t2 are exact (host, fp32) and enter through a 3-row
matmul. Per-core output slab [256, 2048]; per-iteration DMA ~2.13MB is the
roofline (~615GB/s effective), so the kernel sits at the memory ridge.

Per 512-col block, per feature group, the device pipeline is:
  PE : m = (om/2pi)*y + c   one-hot stationary, fp16 exact products
  DVE: r = (m + 1.5*2^23) - 1.5*2^23  dual-op tensor_scalar -> round(m), fp16
  PE : -I matmul accumulates -round(m) into the same PSUM bank -> frac
  ACT: G = sin(2pi*frac) from PSUM -> fp16 SBUF      (Sin valid on [-pi,pi])
then 2 i-tiles x (3-row t12 matmul + 2 group matmuls) accumulate the output
in PSUM, evacuated DVE (it0) / ACT Copy (it1) and DMA'd out. Sin+Copy are
pinned to the trig_and_small table set so exactly one ACT table load happens
per launch. Benchmarking unrolls body_reps bodies inside the For_i repeat
loop: the loop-boundary sync costs tens of us per trip and would otherwise
dominate the steady-state per-iteration time.
"""

from contextlib import ExitStack

import numpy as np

import concourse.bass as bass
import concourse.tile as tile
from concourse import mybir, bacc
from concourse.bass_utils import run_bass_kernel_spmd

AF = mybir.ActivationFunctionType
OP = mybir.AluOpType
F32 = mybir.dt.float32
F16 = mybir.dt.float16

N, M, D = 2048, 2048, 16
NCORES = 8
NS = N // NCORES          # 256 rows of x per core
P = 128
NIT = NS // P             # 2 i-tiles per core
Q = 8                     # cosine terms per coordinate
# Frequency ratios (freq * tmax), optimized offline for p=1.7 with the
# ridge-weighted LSQ objective; weights are re-fit per call for the actual p.
RATIOS = [0.307063, 0.967216, 0.967229, 1.132102,
          2.045687, 2.128786, 3.24151, 4.475632]
NF = D * 2 * Q            # 256 feature rows
NG = NF // P              # 2 groups of 128
JB = 512                  # j-block (one PSUM bank)
NJB = M // JB
MAGIC = float(1.5 * 2 ** 23)
TWO_PI = float(2 * np.pi)

_CACHE = {}


def _patch_act_tables():
    """Keep Sin+Copy in a single table set so the act-table-load pass emits
    exactly one load."""
    if _CACHE.get("patched"):
        return
    import concourse.hw_specs as hw_specs
    import concourse.bacc as bacc_mod

    orig = hw_specs.get_activation_tables
    ours = {AF.Sin, AF.Copy}

    def patched(module_arch):
        tabs = {k: set(v) for k, v in orig(module_arch).items()}
        for name, fns in tabs.items():
            if name != "trig_and_small":
                fns -= ours
        return tabs

    bacc_mod.get_activation_tables = patched
    _CACHE["patched"] = True


def _build_nc(reps=1, body_reps=1):
    _patch_act_tables()
    nc = bacc.Bacc(trn_type="TRN2", target_bir_lowering=False, debug=False,
                   num_devices=NCORES)

    # packed inputs: pk17 = [omg | xt16 | x2t16] on 17 partitions,
    # pk128 = [negI | wvec] on 128, pk3 = [t12s | t12m] on 3.
    pk17 = nc.declare_dram_parameter("pk17", [D + 1, NF + NS + M], F16,
                                     isOutput=False)
    pk128 = nc.declare_dram_parameter("pk128", [P, P], F16, isOutput=False)
    wvec = nc.declare_dram_parameter("wvec", [P, NG], F32, isOutput=False)
    pk3 = nc.declare_dram_parameter("pk3", [3, NS + M], F16, isOutput=False)
    out = nc.declare_dram_parameter("out", [NS, M], F32, isOutput=True)

    with tile.TileContext(nc) as tc, ExitStack() as ctx:
        const = ctx.enter_context(tc.tile_pool(name="const", bufs=1))
        spool = ctx.enter_context(tc.tile_pool(name="s16", bufs=5))
        gpool = ctx.enter_context(tc.tile_pool(name="gfeat", bufs=8))
        opool = ctx.enter_context(tc.tile_pool(name="osb", bufs=2))
        fps = ctx.enter_context(tc.tile_pool(name="fps", bufs=1, space="PSUM"))
        ups = ctx.enter_context(tc.tile_pool(name="ups", bufs=3, space="PSUM"))
        ops = ctx.enter_context(tc.tile_pool(name="ops", bufs=4, space="PSUM"))

        if reps > 1:
            ctx.enter_context(tc.For_i(0, reps, 1, staggered_reset=True))

        for _body in range(body_reps):
            _emit_body(nc, tc, const, spool, gpool, opool, fps, ups, ops,
                       pk17.ap(), pk128.ap(), wvec.ap(), pk3.ap(), out.ap())

    nc.compile()
    return nc


def _emit_body(nc, tc, const, spool, gpool, opool, fps, ups, ops,
               pk17_ap, pk128_ap, wvec_ap, pk3_ap, out_ap):
    # ---- input DMAs (packed: 3 dma_starts instead of 7) ----
    p17 = const.tile([D + 1, NF + NS + M], F16)
    nc.gpsimd.dma_start(out=p17, in_=pk17_ap)
    p128 = const.tile([P, P], F16)
    nc.gpsimd.dma_start(out=p128, in_=pk128_ap)
    wv = const.tile([P, NG], F32)
    nc.gpsimd.dma_start(out=wv, in_=wvec_ap)
    p3 = const.tile([3, NS + M], F16)
    nc.gpsimd.dma_start(out=p3, in_=pk3_ap)
    og = p17[:, 0:NF]
    xt = p17[:, NF:NF + NS]
    x2 = p17[:, NF + NS:NF + NS + M]
    ni = p128[:, 0:P]
    ts_ = p3[:, 0:NS]
    tm = p3[:, NS:NS + M]

    # ---- F (stationary) features: [128, NS] fp16 per group ----
    ffs = []
    for g in range(NG):
        fu = fps.tile([P, NS], F32)
        nc.tensor.matmul(out=fu[:, :], lhsT=og[:, g * P:(g + 1) * P],
                         rhs=xt[:, :], start=True, stop=False)
        sf = spool.tile([P, NS], F16)
        nc.vector.tensor_scalar(out=sf, in0=fu[:, :], scalar1=MAGIC,
                                scalar2=MAGIC, op0=OP.add, op1=OP.subtract)
        nc.tensor.matmul(out=fu[:, :], lhsT=ni, rhs=sf, start=False, stop=True)
        fraw = gpool.tile([P, NS], F16)
        nc.scalar.activation(out=fraw, in_=fu[:, :], func=AF.Sin, scale=TWO_PI)
        ff = const.tile([P, NS], F16, tag=f"ff{g}")
        nc.vector.tensor_scalar(out=ff, in0=fraw, scalar1=wv[:, g:g + 1],
                                scalar2=None, op0=OP.mult)
        ffs.append(ff)

    # ---- main loop over j-blocks ----
    osb = []
    for it in range(NIT):
        ot = opool.tile([P, M], F32, tag=f"osb{it}", name=f"osb{it}")
        osb.append(ot)
    for jb in range(NJB):
        js = slice(jb * JB, (jb + 1) * JB)
        ggs = []
        for g in range(NG):
            u = ups.tile([P, JB], F32)
            nc.tensor.matmul(out=u[:, :], lhsT=og[:, g * P:(g + 1) * P],
                             rhs=x2[:, js], start=True, stop=False)
            s16 = spool.tile([P, JB], F16)
            nc.vector.tensor_scalar(out=s16, in0=u[:, :], scalar1=MAGIC,
                                    scalar2=MAGIC, op0=OP.add, op1=OP.subtract)
            nc.tensor.matmul(out=u[:, :], lhsT=ni, rhs=s16, start=False,
                             stop=True)
            gg = gpool.tile([P, JB], F16)
            nc.scalar.activation(out=gg, in_=u[:, :], func=AF.Sin,
                                 scale=TWO_PI)
            ggs.append(gg)
        for it in range(NIT):
            o = ops.tile([P, JB], F32)
            nc.tensor.matmul(out=o[:, :], lhsT=ts_[:, it * P:(it + 1) * P],
                             rhs=tm[:, js], start=True, stop=False)
            for g in range(NG):
                nc.tensor.matmul(out=o[:, :],
                                 lhsT=ffs[g][:, it * P:(it + 1) * P],
                                 rhs=ggs[g], start=False, stop=(g == NG - 1))
            if it == 0:
                nc.vector.tensor_copy(osb[it][:, js], o[:, :])
            else:
                nc.scalar.activation(out=osb[it][:, js], in_=o[:, :],
                                     func=AF.Copy)

    for it in range(NIT):
        nc.sync.dma_start(out=out_ap[it * P:(it + 1) * P, :], in_=osb[it])


def _get_nc(reps=1, body_reps=1):
    key = ("nc", reps, body_reps)
    if key not in _CACHE:
        _CACHE[key] = _build_nc(reps, body_reps)
    return _CACHE[key]


def _host_prep(x, X2, log_H):
    """Fit the cosine expansion for the runtime p and build all device inputs."""
    x = np.ascontiguousarray(np.asarray(x, dtype=np.float32))
    X2 = np.ascontiguousarray(np.asarray(X2, dtype=np.float32))
    lh = float(np.asarray(log_H, dtype=np.float32))
    H = float(np.logaddexp(0.0, lh))          # softplus
    p = 2.0 * H

    tmax = float(max((x.max(0) - X2.min(0)).max(), (X2.max(0) - x.min(0)).max()))
    tmax = max(tmax, 1e-3)
    qf = np.asarray(RATIOS, dtype=np.float64) / tmax
    qf = np.float16(qf).astype(np.float64)     # fp16-exact frequencies / 2pi
    om = 2 * np.pi * qf
    tg = np.linspace(0, tmax * 1.02, 4000)
    wgt = np.sqrt(np.exp(-tg ** 2 / 4) + 0.02)
    A = np.concatenate([np.ones((len(tg), 1)), np.cos(tg[:, None] * om[None, :])],
                       axis=1)
    Aw = A * wgt[:, None]
    f = tg ** p
    s = max(f.max(), 1e-30)
    coef = np.linalg.solve(Aw.T @ Aw + 1e-2 * np.eye(Q + 1),
                           Aw.T @ ((f / s) * wgt)) * s
    W0, w = coef[0], coef[1:]

    t1 = np.sum(np.abs(x) ** p, axis=1)        # [N]
    t2 = np.sum(np.abs(X2) ** p, axis=1)       # [M]
    c = -0.5 * D * W0
    c_h = float(np.float16(c))

    # feature map: f = g*128 + pp -> (d, q, cs)
    fs = np.arange(NF)
    d_of = fs // (2 * Q)
    r = fs % (2 * Q)
    q_of = r // 2
    cs_of = r % 2

    omg = np.zeros((D + 1, NF), dtype=np.float16)
    omg[d_of, fs] = np.float16(qf[q_of])
    omg[D, fs] = np.float16(0.25 * cs_of)

    pk128 = (-np.eye(P)).astype(np.float16)
    wvec = (-0.5 * w)[q_of].reshape(NG, P).T.astype(np.float32).copy()

    x2t16 = np.ones((D + 1, M), dtype=np.float16)
    x2t16[:D] = np.float16(X2.T)
    t12m = np.ones((3, M), dtype=np.float16)
    t12m[0] = np.float16(0.5 * t2)

    in_maps = []
    for cc in range(NCORES):
        xs = x[cc * NS:(cc + 1) * NS]
        xt16 = np.ones((D + 1, NS), dtype=np.float16)
        xt16[:D] = np.float16(xs.T)
        t12s = np.ones((3, NS), dtype=np.float16)
        t12s[1] = np.float16(0.5 * t1[cc * NS:(cc + 1) * NS] + (c - c_h))
        t12s[2] = np.float16(c_h)
        pk17 = np.concatenate([omg, xt16, x2t16], axis=1)
        pk3 = np.concatenate([t12s, t12m], axis=1)
        in_maps.append({"pk17": pk17, "pk128": pk128, "wvec": wvec,
                        "pk3": pk3})
    return in_maps


def run_spmd(x, X2, log_H, trace=False, reps=1, body_reps=None, **kw):
    """Run the kernel. With explicit ``body_reps``, the device runs ``reps``
    For_i trips of ``body_reps`` unrolled bodies each (reps*body_reps
    iterations total). With ``body_reps=None``, ``reps`` counts total
    iterations and is mapped to trips of 8 automatically. Bodies are unrolled
    inside the loop because the For_i boundary carries an
    InstAllEngineBarrier + semaphore reset whose cost would otherwise
    dominate steady-state per-iteration measurements."""
    if body_reps is None:
        if reps >= 8:
            body_reps, reps = 8, max(1, reps // 8)
        else:
            body_reps = 1
    nc = _get_nc(reps, body_reps)
    in_maps = _host_prep(x, X2, log_H)
    return run_bass_kernel_spmd(nc, in_maps, list(range(NCORES)),
                                trace=trace, **kw)


def kernel(x, X2, log_H):
    res = run_spmd(x, X2, log_H)
    return np.concatenate([res.results[c]["out"] for c in range(NCORES)], axis=0)
